# revision 1
# baseline (speedup 1.0000x reference)
"""RGCN (mean-aggregation) message-passing kernel for 8 Trainium2 NeuronCores.

Problem shapes (hardcoded):
  B=16, L=512, H=256, R=8, E=524288, N = B*2*L = 16384 nodes.

Strategy (dst-sharded, no collectives):
  - Host: node features x = concat(input_s, input_a) -> [N, H]. Edges are
    bucketed by destination-owner core (2048 dst nodes per core), then sorted
    by local segment id  lseg = rel*2048 + local_dst  (relation-major).
    Each core's 16384 segments split into 128 blocks of 128 segments. Every
    block is padded to T_b tiles of 128 edges (uniform static program).
  - Device, per 128-edge tile: indirect-DMA gather of the 128 source rows
    from the replicated HBM node table; build a one-hot selection matrix
    S[p, q] = (slot_p == q) with one DVE is_equal against an iota constant;
    matmul  psum[128 seg, 256 h] += S.T @ G  accumulates the block's
    segment-sum in PSUM. Block result is scaled by 1/count (mean) into SBUF.
  - Mean tiles are PE-transposed, then per-relation GEMMs against W_r plus
    the root GEMM against x^T accumulate out^T [256, 2048] per core.
  - Host reassembles [N, H], splits into (sent, act).
"""

import sys

if "/opt/trn_rl_repo" not in sys.path:
    sys.path.insert(0, "/opt/trn_rl_repo")

import numpy as np

B, L, H, R = 16, 512, 256, 8
N = B * 2 * L          # 16384 nodes
E = 524288
NCORES = 8
NPC = N // NCORES      # 2048 nodes per core
SEGS = NPC * R         # 16384 segments per core
NBLK = SEGS // 128     # 128 blocks per core
P = 128

_COMPILED = {}         # T_b -> (nc, names)


def _build_program(T_b):
    """Build + compile the 8-core SPMD Bass program for tile depth T_b."""
    from concourse import bass, bacc, tile, mybir
    from concourse.masks import make_identity

    f32 = mybir.dt.float32
    i32 = mybir.dt.int32
    T = NBLK * T_b

    nc = bacc.Bacc("TRN2", target_bir_lowering=False, debug=False,
                   num_devices=NCORES)

    bf16 = mybir.dt.bfloat16
    xtab = nc.dram_tensor("xtab", [N, 2 * H], bf16, kind="ExternalInput")
    srcs = nc.dram_tensor("srcs", [P, T], i32, kind="ExternalInput")
    iotad = nc.dram_tensor("iotad", [P, P], f32, kind="ExternalInput")
    identd = nc.dram_tensor("identd", [P, P], f32, kind="ExternalInput")
    slots = nc.dram_tensor("slots", [P, T], f32, kind="ExternalInput")
    recip = nc.dram_tensor("recip", [P, NBLK], f32, kind="ExternalInput")
    wt = nc.dram_tensor("wt", [P, R * 2 * 2 * P], f32, kind="ExternalInput")
    roott = nc.dram_tensor("roott", [P, 2 * 2 * P], f32, kind="ExternalInput")
    biast = nc.dram_tensor("biast", [P, 2], f32, kind="ExternalInput")
    xt = nc.dram_tensor("xt", [P, 2 * NPC], f32, kind="ExternalInput")
    out = nc.dram_tensor("out", [H, NPC], f32, kind="ExternalOutput")

    with tile.TileContext(nc) as tc:
        with (
            tc.tile_pool(name="const", bufs=1) as cpool,
            tc.tile_pool(name="g", bufs=12) as gpool,
            tc.tile_pool(name="s", bufs=12) as spool,
            tc.tile_pool(name="psb", bufs=3, space="PSUM") as psb_pool,
            tc.tile_pool(name="acc", bufs=34) as accpool,
            tc.tile_pool(name="pt", bufs=3, space="PSUM") as pt_pool,
            tc.tile_pool(name="mt", bufs=2) as mtpool,
            tc.tile_pool(name="po", bufs=2, space="PSUM") as po_pool,
        ):
            srcs_sb = cpool.tile([P, T], i32)
            nc.sync.dma_start(srcs_sb[:], srcs.ap())
            slots_sb = cpool.tile([P, T], f32)
            nc.sync.dma_start(slots_sb[:], slots.ap())
            recip_sb = cpool.tile([P, NBLK], f32)
            nc.sync.dma_start(recip_sb[:], recip.ap())
            w_sb = cpool.tile([P, R * 2 * 2 * P], f32)
            nc.sync.dma_start(w_sb[:], wt.ap())
            root_sb = cpool.tile([P, 2 * 2 * P], f32)
            nc.sync.dma_start(root_sb[:], roott.ap())
            bias_sb = cpool.tile([P, 2], f32)
            nc.sync.dma_start(bias_sb[:], biast.ap())
            xt_sb = cpool.tile([P, 2 * NPC], f32)
            nc.sync.dma_start(xt_sb[:], xt.ap())

            iota_f = cpool.tile([P, P], f32)
            nc.sync.dma_start(iota_f[:], iotad.ap())
            ident = cpool.tile([P, P], f32)
            nc.sync.dma_start(ident[:], identd.ap())

            outacc = cpool.tile([P, 2, NPC], f32)

            for r in range(R):
                acc_tiles = []
                for nb in range(16):
                    b = r * 16 + nb
                    ps = psb_pool.tile([P, H], f32)
                    for j in range(T_b):
                        t = b * T_b + j
                        g = gpool.tile([P, 2 * H], bf16)
                        nc.gpsimd.indirect_dma_start(
                            out=g[:], out_offset=None, in_=xtab.ap(),
                            in_offset=bass.IndirectOffsetOnAxis(
                                ap=srcs_sb[:, t:t + 1], axis=0))
                        s = spool.tile([P, P], bf16)
                        nc.vector.tensor_scalar(
                            out=s[:], in0=iota_f[:],
                            scalar1=slots_sb[:, t:t + 1], scalar2=None,
                            op0=mybir.AluOpType.is_equal)
                        # hi + lo bf16 halves accumulate exactly in f32 PSUM
                        nc.tensor.matmul(out=ps[:], lhsT=s[:],
                                         rhs=g[:, 0:H],
                                         start=(j == 0), stop=False)
                        nc.tensor.matmul(out=ps[:], lhsT=s[:],
                                         rhs=g[:, H:2 * H],
                                         start=False, stop=(j == T_b - 1))
                    a = accpool.tile([P, H], f32)
                    nc.vector.tensor_scalar(
                        out=a[:], in0=ps[:], scalar1=recip_sb[:, b:b + 1],
                        scalar2=None, op0=mybir.AluOpType.mult)
                    acc_tiles.append(a)

                # transpose mean_r [2048 n, 256 k] -> mt [128 kpart, 2 kc, 2048 n]
                mt = mtpool.tile([P, 2, NPC], f32)
                for kc in range(2):
                    for nb in range(16):
                        pt = pt_pool.tile([P, P], f32)
                        nc.tensor.transpose(
                            out=pt[:],
                            in_=acc_tiles[nb][:, kc * P:(kc + 1) * P],
                            identity=ident[:])
                        nc.vector.tensor_copy(
                            out=mt[:, kc, nb * P:(nb + 1) * P], in_=pt[:])

                # GEMM: out^T[mc, :] += W_r[:, mc].T-chunks @ mean_r^T
                for mc in range(2):
                    for n4 in range(4):
                        po = po_pool.tile([P, 512], f32)
                        for kc in range(2):
                            wofs = ((r * 2 + kc) * 2 + mc) * P
                            nc.tensor.matmul(
                                out=po[:],
                                lhsT=w_sb[:, wofs:wofs + P],
                                rhs=mt[:, kc, n4 * 512:(n4 + 1) * 512],
                                start=(kc == 0), stop=(kc == 1))
                        osl = outacc[:, mc, n4 * 512:(n4 + 1) * 512]
                        if r == 0:
                            nc.vector.tensor_copy(out=osl, in_=po[:])
                        else:
                            nc.vector.tensor_add(out=osl, in0=osl, in1=po[:])

            # root GEMM: out^T += root^T-chunks @ x^T
            for mc in range(2):
                for n4 in range(4):
                    po = po_pool.tile([P, 512], f32)
                    for kc in range(2):
                        rofs = (kc * 2 + mc) * P
                        nc.tensor.matmul(
                            out=po[:],
                            lhsT=root_sb[:, rofs:rofs + P],
                            rhs=xt_sb[:, kc * NPC + n4 * 512:
                                      kc * NPC + (n4 + 1) * 512],
                            start=(kc == 0), stop=(kc == 1))
                    osl = outacc[:, mc, n4 * 512:(n4 + 1) * 512]
                    nc.vector.tensor_add(out=osl, in0=osl, in1=po[:])

            for mc in range(2):
                nc.vector.tensor_scalar(
                    out=outacc[:, mc, :], in0=outacc[:, mc, :],
                    scalar1=bias_sb[:, mc:mc + 1], scalar2=None,
                    op0=mybir.AluOpType.add)
                nc.sync.dma_start(out.ap()[mc * P:(mc + 1) * P, :],
                                  outacc[:, mc, :])

    nc.compile()
    return nc


def _prep_inputs(input_s, input_a, edge_index, edge_type, weight, root, bias):
    """Host-side sharding/layout prep. Returns (T_b, in_maps)."""
    import ml_dtypes
    x = np.ascontiguousarray(
        np.concatenate([input_s, input_a], axis=1).reshape(N, H)
    ).astype(np.float32)
    x_hi = x.astype(ml_dtypes.bfloat16)
    x_lo = (x - x_hi.astype(np.float32)).astype(ml_dtypes.bfloat16)
    xtab_hl = np.ascontiguousarray(np.concatenate([x_hi, x_lo], axis=1))

    src = np.asarray(edge_index[0]).astype(np.int64)
    dst = np.asarray(edge_index[1]).astype(np.int64)
    et = np.asarray(edge_type).astype(np.int64)

    cnt = np.bincount(dst * R + et, minlength=N * R).reshape(N, R)
    recip_full = (1.0 / np.maximum(cnt, 1)).astype(np.float32)  # [N, R]

    owner = dst // NPC
    lseg = et * NPC + (dst - owner * NPC)          # relation-major local seg
    key = owner * SEGS + lseg
    order = np.argsort(key, kind="stable")
    sk = key[order]
    ssrc = src[order].astype(np.int32)

    bg = sk >> 7                                   # global block id [0, 1024)
    counts_bg = np.bincount(bg, minlength=NCORES * NBLK)
    T_b = int(np.ceil(counts_bg.max() / P))
    cap = T_b * P
    starts = np.concatenate([[0], np.cumsum(counts_bg)])
    pos = np.arange(E) - starts[bg]
    dest = bg * cap + pos

    srcs_pad = np.zeros(NCORES * NBLK * cap, np.int32)
    slots_pad = np.full(NCORES * NBLK * cap, -1.0, np.float32)
    # Dummy (padding) entries sit at each block's tail: encode as -1 so the
    # dma_gather ucode skips them (no descriptor cost). The first 4 blocks of
    # each core keep real row-0 gathers so every G-pool slot's first use
    # fully writes the buffer (a skipped row leaves stale SBUF; stale-NaN * 0
    # in the matmul would poison PSUM).
    srcs_pad[dest] = ssrc
    slots_pad[dest] = (sk & 127).astype(np.float32)
    srcs_c = srcs_pad.reshape(NCORES, NBLK * T_b, P).transpose(0, 2, 1)
    slots_c = slots_pad.reshape(NCORES, NBLK * T_b, P).transpose(0, 2, 1)
    iota_host = np.broadcast_to(np.arange(P, dtype=np.float32), (P, P)).copy()
    ident_host = np.eye(P, dtype=np.float32)

    w_host = np.ascontiguousarray(
        np.asarray(weight, np.float32).reshape(R, 2, P, 2, P)
        .transpose(2, 0, 1, 3, 4).reshape(P, R * 2 * 2 * P))
    root_host = np.ascontiguousarray(
        np.asarray(root, np.float32).reshape(2, P, 2, P)
        .transpose(1, 0, 2, 3).reshape(P, 2 * 2 * P))
    bias_host = np.ascontiguousarray(
        np.asarray(bias, np.float32).reshape(2, P).T)

    in_maps = []
    for c in range(NCORES):
        xc = x[c * NPC:(c + 1) * NPC]              # [2048, 256]
        xt_host = np.ascontiguousarray(
            xc.T.reshape(2, P, NPC).transpose(1, 0, 2).reshape(P, 2 * NPC))
        rc = recip_full[c * NPC:(c + 1) * NPC, :].T.reshape(SEGS)
        recip_host = np.ascontiguousarray(rc.reshape(NBLK, P).T)
        in_maps.append({
            "xtab": xtab_hl,
            "srcs": np.ascontiguousarray(srcs_c[c]),
            "slots": np.ascontiguousarray(slots_c[c]),
            "recip": recip_host,
            "wt": w_host,
            "roott": root_host,
            "biast": bias_host,
            "xt": xt_host,
            "iotad": iota_host,
            "identd": ident_host,
        })
    return T_b, in_maps


def _run(in_maps, T_b, trace=False, trace_cores=None):
    from concourse import bass_utils
    if T_b not in _COMPILED:
        _COMPILED[T_b] = _build_program(T_b)
    nc = _COMPILED[T_b]
    kwargs = {}
    if trace:
        _install_ntff_shim()
        bass_utils.upload_artifacts = lambda tmpdir: tmpdir
        kwargs = dict(trace=True,
                      trace_cores=trace_cores if trace_cores else [0])
    return bass_utils.run_bass_kernel_spmd(
        nc, in_maps, core_ids=list(range(NCORES)), **kwargs)


def _assemble(results):
    full = np.empty((N, H), np.float32)
    for c in range(NCORES):
        full[c * NPC:(c + 1) * NPC, :] = results[c]["out"].T
    dtrp = full.reshape(B, 2 * L, H)
    sent = np.ascontiguousarray(dtrp[:, :L, :])
    act = np.ascontiguousarray(dtrp[:, L:, :])
    return sent, act


def kernel(input_s, input_a, edge_index, edge_type, weight, root, bias,
           _trace=False, _trace_cores=None, _return_stats=False):
    T_b, in_maps = _prep_inputs(input_s, input_a, edge_index, edge_type,
                                weight, root, bias)
    res = _run(in_maps, T_b, trace=_trace, trace_cores=_trace_cores)
    out = _assemble(res.results)
    if _return_stats:
        return out, res
    return out


def _install_ntff_shim():
    """Install antenv.axon_hooks NTFF profiling hook via ctypes (the agent
    image lacks the module; same mechanism trn_boot would use)."""
    import types, ctypes, contextlib
    if "antenv.axon_hooks" in sys.modules:
        return
    so_path = "/opt/axon/libaxon_pjrt.so"
    lib = ctypes.CDLL(so_path)
    if not hasattr(lib, "axon_start_nrt_profile"):
        return
    lib.axon_start_nrt_profile.argtypes = [ctypes.POINTER(ctypes.c_int64),
                                           ctypes.c_size_t]
    lib.axon_start_nrt_profile.restype = ctypes.c_int64
    lib.axon_stop_nrt_profile.argtypes = [ctypes.c_char_p]
    lib.axon_stop_nrt_profile.restype = ctypes.c_int64

    @contextlib.contextmanager
    def _hook(output_dir, device_ids):
        import jax
        jax.devices()
        if device_ids:
            ids = (ctypes.c_int64 * len(device_ids))(*device_ids)
            rc = lib.axon_start_nrt_profile(ids, len(device_ids))
        else:
            rc = lib.axon_start_nrt_profile(None, 0)
        if rc != 0:
            raise RuntimeError(f"axon_start_nrt_profile rc={rc}")
        try:
            yield
        finally:
            n = lib.axon_stop_nrt_profile(str(output_dir).encode())
            if n < 0:
                raise RuntimeError(f"axon_stop_nrt_profile rc={n}")

    import antenv
    mod = types.ModuleType("antenv.axon_hooks")
    mod.get_axon_ntff_profile_hook = lambda: _hook
    mod.set_axon_ntff_profile_hook = lambda h: None
    sys.modules["antenv.axon_hooks"] = mod
    antenv.axon_hooks = mod



# revision 8
# speedup vs baseline: 1.2394x; 1.2394x over previous
"""RGCN (mean-aggregation) message-passing kernel for 8 Trainium2 NeuronCores.

Problem shapes (hardcoded):
  B=16, L=512, H=256, R=8, E=524288, N = B*2*L = 16384 nodes.

Strategy (dst-sharded, no collectives), v2:
  - Host: node features x = concat(input_s, input_a) -> [N, H] fp16 table in
    HBM. Edges are bucketed by destination-owner core (2048 dst nodes per
    core), sorted by local segment id  lseg = rel*2048 + local_dst
    (relation-major). Each core's 16384 segments split into 128 blocks of 128
    segments; every block padded to T_b tiles of 128 edges (uniform static
    program; pad entries gather row 0 with slot=-1 so they contribute 0).
  - Device gather: batched InstDMAGatherAnt (4096 rows x 512B per
    instruction) instead of 640 tiny indirect DMAs -- the ~1us fixed SWDGE
    descriptor-generation cost per instruction was the v1 bottleneck.
  - Aggregation, per 128-edge tile: S[p, q] = (slot_p == q) * recip_p built
    on DVE in batches of 8 tiles (two broadcast tensor_tensor ops);
    matmul(lhsT=G_half, rhs=S) accumulates mean^T [256h, seg] directly in
    PSUM (no PE transposes needed later). PSUM groups of 4 blocks (512 segs)
    are copied to the fp16 mean^T table by the Activation engine.
  - Output: per (mc, n4) chunk, 16 relation GEMMs + 2 root GEMMs chain in a
    single PSUM bank (no vector adds); bias applied during the Activation
    copy-out. out^T [256, 2048] fp32 DMAed per core.
"""

import sys

if "/opt/trn_rl_repo" not in sys.path:
    sys.path.insert(0, "/opt/trn_rl_repo")

import numpy as np

B, L, H, R = 16, 512, 256, 8
N = B * 2 * L          # 16384 nodes
E = 524288
NCORES = 8
NPC = N // NCORES      # 2048 nodes per core
SEGS = NPC * R         # 16384 segments per core
NBLK = SEGS // 128     # 128 blocks per core
P = 128
TPB = 8                # tiles per gather batch (1024 rows; Q7 scratch limit)

_COMPILED = {}         # T_b -> nc


def _build_program(T_b):
    """Build + compile the 8-core SPMD Bass program for tile depth T_b."""
    from concourse import bass, bacc, tile, mybir
    from concourse import library_config

    f32 = mybir.dt.float32
    f16 = mybir.dt.float16
    i16 = mybir.dt.int16
    NT = NBLK * T_b        # total 128-edge tiles
    NB = NT // TPB         # gather batches
    TPG = 4 * T_b          # tiles per psum group (4 blocks = 512 segs)
    NGRP = NBLK // 4       # psum groups

    nc = bacc.Bacc("TRN2", target_bir_lowering=False, debug=False,
                   num_devices=NCORES)

    xtab = nc.dram_tensor("xtab", [N, H], f16, kind="ExternalInput")
    idxsd = nc.dram_tensor("idxsd", [P, NT * 8], i16, kind="ExternalInput")
    slotsd = nc.dram_tensor("slotsd", [P, NT], f16, kind="ExternalInput")
    recipd = nc.dram_tensor("recipd", [P, NT], f16, kind="ExternalInput")
    iotad = nc.dram_tensor("iotad", [P, 8 * P], f16, kind="ExternalInput")
    wt = nc.dram_tensor("wt", [P, R * 2 * 2 * P], f16, kind="ExternalInput")
    roott = nc.dram_tensor("roott", [P, 2 * 2 * P], f16, kind="ExternalInput")
    biast = nc.dram_tensor("biast", [P, 2], f32, kind="ExternalInput")
    xt = nc.dram_tensor("xt", [P, 2 * NPC], f16, kind="ExternalInput")
    out = nc.dram_tensor("out", [2, P, NPC], f32, kind="ExternalOutput")

    with tile.TileContext(nc) as tc:
        with (
            tc.tile_pool(name="const", bufs=1) as cpool,
            tc.tile_pool(name="g", bufs=6) as gpool,
            tc.tile_pool(name="s", bufs=4) as spool,
            tc.tile_pool(name="pt", bufs=2, space="PSUM") as pt_pool,
            tc.tile_pool(name="po", bufs=2, space="PSUM") as po_pool,
        ):
            idx_sb = cpool.tile([P, NT * 8], i16)
            nc.sync.dma_start(idx_sb[:], idxsd.ap())
            slots_sb = cpool.tile([P, NT, 1], f16)
            nc.sync.dma_start(slots_sb[:], slotsd.ap())
            recip_sb = cpool.tile([P, NT, 1], f16)
            nc.sync.dma_start(recip_sb[:], recipd.ap())
            iota_sb = cpool.tile([P, 8, P], f16)
            nc.sync.dma_start(iota_sb[:], iotad.ap())
            w_sb = cpool.tile([P, R * 2 * 2 * P], f16)
            nc.sync.dma_start(w_sb[:], wt.ap())
            root_sb = cpool.tile([P, 2 * 2 * P], f16)
            nc.sync.dma_start(root_sb[:], roott.ap())
            bias_sb = cpool.tile([P, 2], f32)
            nc.sync.dma_start(bias_sb[:], biast.ap())
            xt_sb = cpool.tile([P, 2, NPC], f16)
            nc.sync.dma_start(xt_sb[:], xt.ap())

            mt = cpool.tile([P, 2, SEGS], f16)      # mean^T, all relations
            out_sb = cpool.tile([P, 2, NPC], f32)

            nc.gpsimd.load_library(library_config.mlp)

            g_tiles = []
            for b in range(NB):
                g = gpool.tile([P, TPB, H], f16)
                nc.gpsimd.dma_gather(
                    g[:], xtab.ap(), idx_sb[:, b * TPB * 8:(b + 1) * TPB * 8],
                    num_idxs=TPB * P, num_idxs_reg=TPB * P, elem_size=H)
                g_tiles.append(g)

            s8 = None
            psT = None
            for t in range(NT):
                if t % 8 == 0:
                    g8 = t // 8
                    s8 = spool.tile([P, 8, P], f16)
                    nc.vector.tensor_tensor(
                        out=s8[:], in0=iota_sb[:],
                        in1=slots_sb[:, g8 * 8:(g8 + 1) * 8, :]
                        .to_broadcast([P, 8, P]),
                        op=mybir.AluOpType.is_equal)
                    nc.vector.tensor_tensor(
                        out=s8[:], in0=s8[:],
                        in1=recip_sb[:, g8 * 8:(g8 + 1) * 8, :]
                        .to_broadcast([P, 8, P]),
                        op=mybir.AluOpType.mult)
                blk = t // T_b
                grp = blk // 4
                pos = blk % 4
                if t % TPG == 0:
                    psT = [pt_pool.tile([P, 512], f32, name=f"psT{kc}")
                           for kc in range(2)]
                gt = g_tiles[t // TPB]
                for kc in range(2):
                    nc.tensor.matmul(
                        out=psT[kc][:, pos * P:(pos + 1) * P],
                        lhsT=gt[:, t % TPB, kc * P:(kc + 1) * P],
                        rhs=s8[:, t % 8, :],
                        start=(t % T_b == 0), stop=(t % T_b == T_b - 1))
                if (t + 1) % TPG == 0:
                    for kc in range(2):
                        nc.scalar.copy(
                            out=mt[:, kc, grp * 512:(grp + 1) * 512],
                            in_=psT[kc][:])

            # final GEMM: out^T[mc] = sum_r W_r^T-chunks @ mean_r^T
            #                        + root^T-chunks @ x^T  (+ bias)
            for mc in range(2):
                for n4 in range(4):
                    po = po_pool.tile([P, 512], f32)
                    for r in range(R):
                        for kc in range(2):
                            wofs = ((r * 2 + kc) * 2 + mc) * P
                            nc.tensor.matmul(
                                out=po[:],
                                lhsT=w_sb[:, wofs:wofs + P],
                                rhs=mt[:, kc, r * NPC + n4 * 512:
                                       r * NPC + (n4 + 1) * 512],
                                start=(r == 0 and kc == 0), stop=False)
                    for kc in range(2):
                        rofs = (kc * 2 + mc) * P
                        nc.tensor.matmul(
                            out=po[:],
                            lhsT=root_sb[:, rofs:rofs + P],
                            rhs=xt_sb[:, kc, n4 * 512:(n4 + 1) * 512],
                            start=False, stop=(kc == 1))
                    nc.vector.tensor_scalar(
                        out=out_sb[:, mc, n4 * 512:(n4 + 1) * 512],
                        in0=po[:], scalar1=bias_sb[:, mc:mc + 1],
                        scalar2=None, op0=mybir.AluOpType.add)
            for mc in range(2):
                nc.sync.dma_start(out.ap()[mc], out_sb[:, mc, :])

    nc.compile()
    return nc


def _prep_inputs(input_s, input_a, edge_index, edge_type, weight, root, bias):
    """Host-side sharding/layout prep. Returns (T_b, in_maps)."""
    import ml_dtypes
    x = np.ascontiguousarray(
        np.concatenate([input_s, input_a], axis=1).reshape(N, H)
    ).astype(np.float32)
    xtab = x.astype(np.float16)

    src = np.asarray(edge_index[0]).astype(np.int64)
    dst = np.asarray(edge_index[1]).astype(np.int64)
    et = np.asarray(edge_type).astype(np.int64)

    cnt = np.bincount(dst * R + et, minlength=N * R).reshape(N, R)
    recip_full = (1.0 / np.maximum(cnt, 1)).astype(np.float32)  # [N, R]

    owner = dst // NPC
    lseg = et * NPC + (dst - owner * NPC)          # relation-major local seg
    key = owner * SEGS + lseg
    order = np.argsort(key, kind="stable")
    sk = key[order]
    ssrc = src[order].astype(np.int16)
    srecip = recip_full[dst[order], et[order]].astype(np.float16)

    bg = sk >> 7                                   # global block id [0, 1024)
    counts_bg = np.bincount(bg, minlength=NCORES * NBLK)
    T_b = int(np.ceil(counts_bg.max() / P))
    cap = T_b * P
    NT = NBLK * T_b
    starts = np.concatenate([[0], np.cumsum(counts_bg)])
    pos = np.arange(E) - starts[bg]
    dest = bg * cap + pos

    # pad entries gather row 0 (valid descriptor; slot=-1 zeroes them in S)
    srcs_pad = np.zeros(NCORES * NBLK * cap, np.int16)
    slots_pad = np.full(NCORES * NBLK * cap, -1.0, np.float16)
    recip_pad = np.zeros(NCORES * NBLK * cap, np.float16)
    srcs_pad[dest] = ssrc
    slots_pad[dest] = (sk & 127).astype(np.float16)
    recip_pad[dest] = srecip

    srcs_c = srcs_pad.reshape(NCORES, NT * P)
    slots_c = slots_pad.reshape(NCORES, NT, P)
    recip_c = recip_pad.reshape(NCORES, NT, P)
    iota_host = np.tile(np.arange(P, dtype=np.float16), (P, 8, 1)
                        ).reshape(P, 8 * P)

    w_host = np.ascontiguousarray(
        np.asarray(weight, np.float32).reshape(R, 2, P, 2, P)
        .transpose(2, 0, 1, 3, 4).reshape(P, R * 2 * 2 * P)).astype(np.float16)
    root_host = np.ascontiguousarray(
        np.asarray(root, np.float32).reshape(2, P, 2, P)
        .transpose(1, 0, 2, 3).reshape(P, 2 * 2 * P)).astype(np.float16)
    bias_host = np.ascontiguousarray(
        np.asarray(bias, np.float32).reshape(2, P).T)

    in_maps = []
    for c in range(NCORES):
        xc = x[c * NPC:(c + 1) * NPC]              # [2048, 256]
        xt_host = np.ascontiguousarray(
            xc.T.reshape(2, P, NPC).transpose(1, 0, 2).reshape(P, 2 * NPC)
        ).astype(np.float16)
        idx_host = np.ascontiguousarray(
            np.tile(srcs_c[c].reshape(NT * 8, 16).T, (8, 1)))
        in_maps.append({
            "xtab": xtab,
            "idxsd": idx_host,
            "slotsd": np.ascontiguousarray(slots_c[c].T),
            "recipd": np.ascontiguousarray(recip_c[c].T),
            "iotad": iota_host,
            "wt": w_host,
            "roott": root_host,
            "biast": bias_host,
            "xt": xt_host,
        })
    return T_b, in_maps


def _run(in_maps, T_b, trace=False, trace_cores=None):
    from concourse import bass_utils
    if T_b not in _COMPILED:
        _COMPILED[T_b] = _build_program(T_b)
    nc = _COMPILED[T_b]
    kwargs = {}
    if trace:
        _install_ntff_shim()
        bass_utils.upload_artifacts = lambda tmpdir: tmpdir
        kwargs = dict(trace=True,
                      trace_cores=trace_cores if trace_cores else [0])
    return bass_utils.run_bass_kernel_spmd(
        nc, in_maps, core_ids=list(range(NCORES)), **kwargs)


def _assemble(results):
    full = np.empty((N, H), np.float32)
    for c in range(NCORES):
        o = results[c]["out"]                      # [2, 128, 2048]
        full[c * NPC:(c + 1) * NPC, 0:P] = o[0].T
        full[c * NPC:(c + 1) * NPC, P:2 * P] = o[1].T
    dtrp = full.reshape(B, 2 * L, H)
    sent = np.ascontiguousarray(dtrp[:, :L, :])
    act = np.ascontiguousarray(dtrp[:, L:, :])
    return sent, act


def kernel(input_s, input_a, edge_index, edge_type, weight, root, bias,
           _trace=False, _trace_cores=None, _return_stats=False):
    T_b, in_maps = _prep_inputs(input_s, input_a, edge_index, edge_type,
                                weight, root, bias)
    res = _run(in_maps, T_b, trace=_trace, trace_cores=_trace_cores)
    out = _assemble(res.results)
    if _return_stats:
        return out, res
    return out


def _install_ntff_shim():
    """Install antenv.axon_hooks NTFF profiling hook via ctypes (the agent
    image lacks the module; same mechanism trn_boot would use)."""
    import types, ctypes, contextlib
    if "antenv.axon_hooks" in sys.modules:
        return
    so_path = "/opt/axon/libaxon_pjrt.so"
    lib = ctypes.CDLL(so_path)
    if not hasattr(lib, "axon_start_nrt_profile"):
        return
    lib.axon_start_nrt_profile.argtypes = [ctypes.POINTER(ctypes.c_int64),
                                           ctypes.c_size_t]
    lib.axon_start_nrt_profile.restype = ctypes.c_int64
    lib.axon_stop_nrt_profile.argtypes = [ctypes.c_char_p]
    lib.axon_stop_nrt_profile.restype = ctypes.c_int64

    @contextlib.contextmanager
    def _hook(output_dir, device_ids):
        import jax
        jax.devices()
        if device_ids:
            ids = (ctypes.c_int64 * len(device_ids))(*device_ids)
            rc = lib.axon_start_nrt_profile(ids, len(device_ids))
        else:
            rc = lib.axon_start_nrt_profile(None, 0)
        if rc != 0:
            raise RuntimeError(f"axon_start_nrt_profile rc={rc}")
        try:
            yield
        finally:
            n = lib.axon_stop_nrt_profile(str(output_dir).encode())
            if n < 0:
                raise RuntimeError(f"axon_stop_nrt_profile rc={n}")

    import antenv
    mod = types.ModuleType("antenv.axon_hooks")
    mod.get_axon_ntff_profile_hook = lambda: _hook
    mod.set_axon_ntff_profile_hook = lambda h: None
    sys.modules["antenv.axon_hooks"] = mod
    antenv.axon_hooks = mod


# revision 17
# speedup vs baseline: 2.8886x; 2.3306x over previous
"""RGCN (mean-aggregation) message-passing kernel for 8 Trainium2 NeuronCores.

Problem shapes (hardcoded):
  B=16, L=512, H=256, R=8, E=524288, N = B*2*L = 16384 nodes.

Strategy (dst-sharded, no collectives), v2:
  - Host: node features x = concat(input_s, input_a) -> [N, H] fp16 table in
    HBM. Edges are bucketed by destination-owner core (2048 dst nodes per
    core), sorted by local segment id  lseg = rel*2048 + local_dst
    (relation-major). Each core's 16384 segments split into 128 blocks of 128
    segments; every block padded to T_b tiles of 128 edges (uniform static
    program; pad entries gather row 0 with slot=-1 so they contribute 0).
  - Device gather: batched InstDMAGatherAnt (4096 rows x 512B per
    instruction) instead of 640 tiny indirect DMAs -- the ~1us fixed SWDGE
    descriptor-generation cost per instruction was the v1 bottleneck.
  - Aggregation, per 128-edge tile: S[p, q] = (slot_p == q) * recip_p built
    on DVE in batches of 8 tiles (two broadcast tensor_tensor ops);
    matmul(lhsT=G_half, rhs=S) accumulates mean^T [256h, seg] directly in
    PSUM (no PE transposes needed later). PSUM groups of 4 blocks (512 segs)
    are copied to the fp16 mean^T table by the Activation engine.
  - Output: per (mc, n4) chunk, 16 relation GEMMs + 2 root GEMMs chain in a
    single PSUM bank (no vector adds); bias applied during the Activation
    copy-out. out^T [256, 2048] fp32 DMAed per core.
"""

import sys

if "/opt/trn_rl_repo" not in sys.path:
    sys.path.insert(0, "/opt/trn_rl_repo")

import numpy as np

B, L, H, R = 16, 512, 256, 8
N = B * 2 * L          # 16384 nodes
E = 524288
NCORES = 8
NPC = N // NCORES      # 2048 nodes per core
SEGS = NPC * R         # 16384 segments per core
NBLK = SEGS // 128     # 128 blocks per core
P = 128
TPB = 8                # tiles per gather batch (1024 rows; Q7 scratch limit)

_COMPILED = {}         # T_b -> nc


def _build_program(T_b):
    """Build + compile the 8-core SPMD Bass program for tile depth T_b."""
    from concourse import bass, bacc, tile, mybir
    from concourse import library_config

    f32 = mybir.dt.float32
    f16 = mybir.dt.float16
    i16 = mybir.dt.int16
    NT = NBLK * T_b        # total 128-edge tiles
    NB = NT // TPB         # gather batches
    TPG = 4 * T_b          # tiles per psum group (4 blocks = 512 segs)
    NGRP = NBLK // 4       # psum groups

    nc = bacc.Bacc("TRN2", target_bir_lowering=False, debug=False,
                   num_devices=NCORES, num_swdge_queues=4)

    xtab = nc.dram_tensor("xtab", [N, H], f16, kind="ExternalInput")
    idxsd = nc.dram_tensor("idxsd", [P, NT * 8], i16, kind="ExternalInput")
    slotsd = nc.dram_tensor("slotsd", [P, NT], f16, kind="ExternalInput")
    recipd = nc.dram_tensor("recipd", [P, SEGS], f16, kind="ExternalInput")
    iotad = nc.dram_tensor("iotad", [P, 8 * P], f16, kind="ExternalInput")
    wt = nc.dram_tensor("wt", [P, R * 2 * 2 * P], f16, kind="ExternalInput")
    roott = nc.dram_tensor("roott", [P, 2 * 2 * P], f16, kind="ExternalInput")
    biast = nc.dram_tensor("biast", [P, 2], f32, kind="ExternalInput")
    xt = nc.dram_tensor("xt", [P, 2 * NPC], f16, kind="ExternalInput")
    out = nc.dram_tensor("out", [2, P, NPC], f32, kind="ExternalOutput")

    with tile.TileContext(nc) as tc:
        with (
            tc.tile_pool(name="const", bufs=1) as cpool,
            tc.tile_pool(name="g", bufs=6) as gpool,
            tc.tile_pool(name="s", bufs=4) as spool,
            tc.tile_pool(name="pt", bufs=2, space="PSUM") as pt_pool,
            tc.tile_pool(name="po", bufs=2, space="PSUM") as po_pool,
        ):
            idx_sb = cpool.tile([P, NT * 8], i16)
            nc.sync.dma_start(idx_sb[:], idxsd.ap())
            slots_sb = cpool.tile([P, NT, 1], f16)
            nc.sync.dma_start(slots_sb[:], slotsd.ap())
            recip_sb = cpool.tile([P, SEGS], f16)
            nc.sync.dma_start(recip_sb[:], recipd.ap())
            iota_sb = cpool.tile([P, 8, P], f16)
            nc.sync.dma_start(iota_sb[:], iotad.ap())
            w_sb = cpool.tile([P, R * 2 * 2 * P], f16)
            nc.sync.dma_start(w_sb[:], wt.ap())
            root_sb = cpool.tile([P, 2 * 2 * P], f16)
            nc.sync.dma_start(root_sb[:], roott.ap())
            bias_sb = cpool.tile([P, 2], f32)
            nc.sync.dma_start(bias_sb[:], biast.ap())
            xt_sb = cpool.tile([P, 2, NPC], f16)
            nc.sync.dma_start(xt_sb[:], xt.ap())

            mt = cpool.tile([P, 2, SEGS], f16)      # mean^T, all relations
            out_sb = cpool.tile([P, 2, NPC], f32)

            nc.gpsimd.load_library(library_config.mlp)

            g_tiles = []
            for b in range(NB):
                g = gpool.tile([P, TPB, H], f16)
                nc.gpsimd.dma_gather(
                    g[:], xtab.ap(), idx_sb[:, b * TPB * 8:(b + 1) * TPB * 8],
                    num_idxs=TPB * P, num_idxs_reg=TPB * P, elem_size=H,
                    queue_num=b % 4)
                g_tiles.append(g)

            s8 = None
            psT = None
            for t in range(NT):
                if t % 8 == 0:
                    g8 = t // 8
                    s8 = spool.tile([P, 8, P], f16)
                    nc.vector.tensor_tensor(
                        out=s8[:], in0=iota_sb[:],
                        in1=slots_sb[:, g8 * 8:(g8 + 1) * 8, :]
                        .to_broadcast([P, 8, P]),
                        op=mybir.AluOpType.is_equal)
                blk = t // T_b
                grp = blk // 4
                pos = blk % 4
                if t % TPG == 0:
                    psT = [pt_pool.tile([P, 512], f32, name=f"psT{kc}")
                           for kc in range(2)]
                gt = g_tiles[t // TPB]
                for kc in range(2):
                    nc.tensor.matmul(
                        out=psT[kc][:, pos * P:(pos + 1) * P],
                        lhsT=gt[:, t % TPB, kc * P:(kc + 1) * P],
                        rhs=s8[:, t % 8, :],
                        start=(t % T_b == 0), stop=(t % T_b == T_b - 1))
                if (t + 1) % TPG == 0:
                    # mean = sum * recip, fused into the PSUM->SBUF copy
                    for kc in range(2):
                        nc.vector.tensor_tensor(
                            out=mt[:, kc, grp * 512:(grp + 1) * 512],
                            in0=psT[kc][:],
                            in1=recip_sb[:, grp * 512:(grp + 1) * 512],
                            op=mybir.AluOpType.mult)

            # final GEMM: out^T[mc] = sum_r W_r^T-chunks @ mean_r^T
            #                        + root^T-chunks @ x^T  (+ bias)
            for mc in range(2):
                for n4 in range(4):
                    po = po_pool.tile([P, 512], f32)
                    for r in range(R):
                        for kc in range(2):
                            wofs = ((r * 2 + kc) * 2 + mc) * P
                            nc.tensor.matmul(
                                out=po[:],
                                lhsT=w_sb[:, wofs:wofs + P],
                                rhs=mt[:, kc, r * NPC + n4 * 512:
                                       r * NPC + (n4 + 1) * 512],
                                start=(r == 0 and kc == 0), stop=False)
                    for kc in range(2):
                        rofs = (kc * 2 + mc) * P
                        nc.tensor.matmul(
                            out=po[:],
                            lhsT=root_sb[:, rofs:rofs + P],
                            rhs=xt_sb[:, kc, n4 * 512:(n4 + 1) * 512],
                            start=False, stop=(kc == 1))
                    nc.vector.tensor_scalar(
                        out=out_sb[:, mc, n4 * 512:(n4 + 1) * 512],
                        in0=po[:], scalar1=bias_sb[:, mc:mc + 1],
                        scalar2=None, op0=mybir.AluOpType.add)
            for mc in range(2):
                nc.sync.dma_start(out.ap()[mc], out_sb[:, mc, :])

    nc.compile()
    return nc


def _prep_inputs(input_s, input_a, edge_index, edge_type, weight, root, bias):
    """Host-side sharding/layout prep. Returns (T_b, in_maps)."""
    import ml_dtypes
    x = np.ascontiguousarray(
        np.concatenate([input_s, input_a], axis=1).reshape(N, H)
    ).astype(np.float32)
    xtab = x.astype(np.float16)

    src = np.asarray(edge_index[0]).astype(np.int64)
    dst = np.asarray(edge_index[1]).astype(np.int64)
    et = np.asarray(edge_type).astype(np.int64)

    cnt = np.bincount(dst * R + et, minlength=N * R).reshape(N, R)
    recip_full = (1.0 / np.maximum(cnt, 1)).astype(np.float32)  # [N, R]

    owner = dst // NPC
    lseg = et * NPC + (dst - owner * NPC)          # relation-major local seg
    key = owner * SEGS + lseg
    order = np.argsort(key, kind="stable")
    sk = key[order]
    ssrc = src[order].astype(np.int16)

    bg = sk >> 7                                   # global block id [0, 1024)
    counts_bg = np.bincount(bg, minlength=NCORES * NBLK)
    T_b = int(np.ceil(counts_bg.max() / P))
    cap = T_b * P
    NT = NBLK * T_b
    starts = np.concatenate([[0], np.cumsum(counts_bg)])
    pos = np.arange(E) - starts[bg]
    dest = bg * cap + pos

    # pad entries gather row 0 (valid descriptor; slot=-1 zeroes them in S)
    srcs_pad = np.zeros(NCORES * NBLK * cap, np.int16)
    slots_pad = np.full(NCORES * NBLK * cap, -1.0, np.float16)
    srcs_pad[dest] = ssrc
    slots_pad[dest] = (sk & 127).astype(np.float16)

    srcs_c = srcs_pad.reshape(NCORES, NT * P)
    slots_c = slots_pad.reshape(NCORES, NT, P)
    iota_host = np.tile(np.arange(P, dtype=np.float16), (P, 8, 1)
                        ).reshape(P, 8 * P)

    w_host = np.ascontiguousarray(
        np.asarray(weight, np.float32).reshape(R, 2, P, 2, P)
        .transpose(2, 0, 1, 3, 4).reshape(P, R * 2 * 2 * P)).astype(np.float16)
    root_host = np.ascontiguousarray(
        np.asarray(root, np.float32).reshape(2, P, 2, P)
        .transpose(1, 0, 2, 3).reshape(P, 2 * 2 * P)).astype(np.float16)
    bias_host = np.ascontiguousarray(
        np.asarray(bias, np.float32).reshape(2, P).T)

    in_maps = []
    for c in range(NCORES):
        xc = x[c * NPC:(c + 1) * NPC]              # [2048, 256]
        xt_host = np.ascontiguousarray(
            xc.T.reshape(2, P, NPC).transpose(1, 0, 2).reshape(P, 2 * NPC)
        ).astype(np.float16)
        idx_host = np.ascontiguousarray(
            np.tile(srcs_c[c].reshape(NT * 8, 16).T, (8, 1)))
        rc = recip_full[c * NPC:(c + 1) * NPC, :].T.reshape(SEGS)
        recip_host = np.ascontiguousarray(
            np.broadcast_to(rc.astype(np.float16), (P, SEGS)))
        in_maps.append({
            "xtab": xtab,
            "idxsd": idx_host,
            "slotsd": np.ascontiguousarray(slots_c[c].T),
            "recipd": recip_host,
            "iotad": iota_host,
            "wt": w_host,
            "roott": root_host,
            "biast": bias_host,
            "xt": xt_host,
        })
    return T_b, in_maps


def _run(in_maps, T_b, trace=False, trace_cores=None):
    from concourse import bass_utils
    if T_b not in _COMPILED:
        _COMPILED[T_b] = _build_program(T_b)
    nc = _COMPILED[T_b]
    kwargs = {}
    if trace:
        _install_ntff_shim()
        bass_utils.upload_artifacts = lambda tmpdir: tmpdir
        kwargs = dict(trace=True,
                      trace_cores=trace_cores if trace_cores else [0])
    return bass_utils.run_bass_kernel_spmd(
        nc, in_maps, core_ids=list(range(NCORES)), **kwargs)


def _assemble(results):
    full = np.empty((N, H), np.float32)
    for c in range(NCORES):
        o = results[c]["out"]                      # [2, 128, 2048]
        full[c * NPC:(c + 1) * NPC, 0:P] = o[0].T
        full[c * NPC:(c + 1) * NPC, P:2 * P] = o[1].T
    dtrp = full.reshape(B, 2 * L, H)
    sent = np.ascontiguousarray(dtrp[:, :L, :])
    act = np.ascontiguousarray(dtrp[:, L:, :])
    return sent, act


def kernel(input_s, input_a, edge_index, edge_type, weight, root, bias,
           _trace=False, _trace_cores=None, _return_stats=False):
    T_b, in_maps = _prep_inputs(input_s, input_a, edge_index, edge_type,
                                weight, root, bias)
    res = _run(in_maps, T_b, trace=_trace, trace_cores=_trace_cores)
    out = _assemble(res.results)
    if _return_stats:
        return out, res
    return out


def _install_ntff_shim():
    """Install antenv.axon_hooks NTFF profiling hook via ctypes (the agent
    image lacks the module; same mechanism trn_boot would use)."""
    import types, ctypes, contextlib
    if "antenv.axon_hooks" in sys.modules:
        return
    so_path = "/opt/axon/libaxon_pjrt.so"
    lib = ctypes.CDLL(so_path)
    if not hasattr(lib, "axon_start_nrt_profile"):
        return
    lib.axon_start_nrt_profile.argtypes = [ctypes.POINTER(ctypes.c_int64),
                                           ctypes.c_size_t]
    lib.axon_start_nrt_profile.restype = ctypes.c_int64
    lib.axon_stop_nrt_profile.argtypes = [ctypes.c_char_p]
    lib.axon_stop_nrt_profile.restype = ctypes.c_int64

    @contextlib.contextmanager
    def _hook(output_dir, device_ids):
        import jax
        jax.devices()
        if device_ids:
            ids = (ctypes.c_int64 * len(device_ids))(*device_ids)
            rc = lib.axon_start_nrt_profile(ids, len(device_ids))
        else:
            rc = lib.axon_start_nrt_profile(None, 0)
        if rc != 0:
            raise RuntimeError(f"axon_start_nrt_profile rc={rc}")
        try:
            yield
        finally:
            n = lib.axon_stop_nrt_profile(str(output_dir).encode())
            if n < 0:
                raise RuntimeError(f"axon_stop_nrt_profile rc={n}")

    import antenv
    mod = types.ModuleType("antenv.axon_hooks")
    mod.get_axon_ntff_profile_hook = lambda: _hook
    mod.set_axon_ntff_profile_hook = lambda h: None
    sys.modules["antenv.axon_hooks"] = mod
    antenv.axon_hooks = mod
